# revision 1
# baseline (speedup 1.0000x reference)
"""Trainium2 Bass kernel for the e3nn-style concat + per-irrep Linear problem.

Reference computation (N = 200000 nodes, 480-dim features per input):
  per input: 128x0e (dims 0:128) + 64x1e (dims 128:320) + 32x2e (dims 320:480)
  s = [s1, s2] @ W0 * inv0 + b0        # [N, 128]
  v = einsum('nmi,mo->noi', [v1,v2], W1) * inv1   # [N, 64, 3]
  t = einsum('nmi,mo->noi', [t1,t2], W2) * inv2   # [N, 32, 5]
  out = concat([s, v.flat, t.flat], axis=1)       # [N, 480]

Strategy (memory-bound, data-parallel over nodes across 8 cores):
  - Host: repack both inputs into one channel-major tensor XR [960, N]
    (irrep components de-interleaved so every matmul contraction slab is a
    contiguous 128/64-partition block), fold the 1/sqrt(K) norms into the
    weights, pair up the five l=2 components into block-diagonal weights.
  - Device: per 512-node block, 8 contiguous DMA loads, 8 fp32 matmuls with
    stationary weights (streaming 512 node-columns), bias-add/copy
    PSUM->SBUF, 4 contiguous DMA stores of the channel-major output
    outT [480, N].
  - Host: transpose/interleave outT back to the reference layout.
"""

import numpy as np

MUL0, MUL1, MUL2 = 128, 64, 32
N_TOTAL = 200000
N_CORES = 8
NC_NODES = N_TOTAL // N_CORES          # 25000
NODE_BLOCK = 512
N_BLOCKS = (NC_NODES + NODE_BLOCK - 1) // NODE_BLOCK   # 49
NPAD = N_BLOCKS * NODE_BLOCK           # 25088

_PROGRAM_CACHE = {}


def _build_program(npad):
    import concourse.mybir as mybir
    from concourse import bacc
    import concourse.tile as tile

    f32 = mybir.dt.float32
    nc = bacc.Bacc("TRN2", target_bir_lowering=False, debug=False)

    xin = nc.dram_tensor("xin", [960, npad], f32, kind="ExternalInput").ap()
    w0a = nc.dram_tensor("w0a", [128, 128], f32, kind="ExternalInput").ap()
    w0b = nc.dram_tensor("w0b", [128, 128], f32, kind="ExternalInput").ap()
    w1d = nc.dram_tensor("w1d", [128, 64], f32, kind="ExternalInput").ap()
    w2p = nc.dram_tensor("w2p", [128, 64], f32, kind="ExternalInput").ap()
    w2s = nc.dram_tensor("w2s", [64, 32], f32, kind="ExternalInput").ap()
    b0d = nc.dram_tensor("b0d", [128, 1], f32, kind="ExternalInput").ap()
    outT = nc.dram_tensor("outT", [480, npad], f32, kind="ExternalOutput").ap()

    nblocks = npad // NODE_BLOCK

    with tile.TileContext(nc) as tc:
        with (
            tc.tile_pool(name="wpool", bufs=1) as wpool,
            tc.tile_pool(name="inpool", bufs=3) as inpool,
            tc.tile_pool(name="psum", bufs=2, space="PSUM") as psum,
            tc.tile_pool(name="outpool", bufs=3) as outpool,
        ):
            wa_t = wpool.tile([128, 128], f32)
            wb_t = wpool.tile([128, 128], f32)
            w1_t = wpool.tile([128, 64], f32)
            w2p_t = wpool.tile([128, 64], f32)
            w2s_t = wpool.tile([64, 32], f32)
            b0_t = wpool.tile([128, 1], f32)
            nc.sync.dma_start(wa_t[:], w0a)
            nc.sync.dma_start(wb_t[:], w0b)
            nc.sync.dma_start(w1_t[:], w1d)
            nc.sync.dma_start(w2p_t[:], w2p)
            nc.sync.dma_start(w2s_t[:], w2s)
            nc.sync.dma_start(b0_t[:], b0d)

            for blk in range(nblocks):
                sl = slice(blk * NODE_BLOCK, (blk + 1) * NODE_BLOCK)

                # Input slabs (channel-major, K on partitions, nodes on free)
                t_s1 = inpool.tile([128, NODE_BLOCK], f32)
                t_s2 = inpool.tile([128, NODE_BLOCK], f32)
                t_v0 = inpool.tile([128, NODE_BLOCK], f32)
                t_v1 = inpool.tile([128, NODE_BLOCK], f32)
                t_v2 = inpool.tile([128, NODE_BLOCK], f32)
                t_tp0 = inpool.tile([128, NODE_BLOCK], f32)
                t_tp1 = inpool.tile([128, NODE_BLOCK], f32)
                t_t4 = inpool.tile([64, NODE_BLOCK], f32)
                nc.sync.dma_start(t_s1[:], xin[0:128, sl])
                nc.sync.dma_start(t_s2[:], xin[128:256, sl])
                nc.sync.dma_start(t_v0[:], xin[256:384, sl])
                nc.sync.dma_start(t_v1[:], xin[384:512, sl])
                nc.sync.dma_start(t_v2[:], xin[512:640, sl])
                nc.sync.dma_start(t_tp0[:], xin[640:768, sl])
                nc.sync.dma_start(t_tp1[:], xin[768:896, sl])
                nc.sync.dma_start(t_t4[:], xin[896:960, sl])

                # out^T rows: [s(128)] [v0|v1] [v2|t0,t1] [t2,t3|t4]
                p0 = psum.tile([128, NODE_BLOCK], f32)
                p1 = psum.tile([128, NODE_BLOCK], f32)
                p2 = psum.tile([128, NODE_BLOCK], f32)
                p3 = psum.tile([96, NODE_BLOCK], f32)

                nc.tensor.matmul(p0[:], wa_t[:], t_s1[:], start=True, stop=False)
                nc.tensor.matmul(p0[:], wb_t[:], t_s2[:], start=False, stop=True)
                nc.tensor.matmul(p1[0:64, :], w1_t[:], t_v0[:])
                nc.tensor.matmul(p1[64:128, :], w1_t[:], t_v1[:])
                nc.tensor.matmul(p2[0:64, :], w1_t[:], t_v2[:])
                nc.tensor.matmul(p2[64:128, :], w2p_t[:], t_tp0[:])
                nc.tensor.matmul(p3[0:64, :], w2p_t[:], t_tp1[:])
                nc.tensor.matmul(p3[64:96, :], w2s_t[:], t_t4[:])

                o0 = outpool.tile([128, NODE_BLOCK], f32)
                o1 = outpool.tile([128, NODE_BLOCK], f32)
                o2 = outpool.tile([128, NODE_BLOCK], f32)
                o3 = outpool.tile([96, NODE_BLOCK], f32)
                nc.vector.tensor_scalar_add(o0[:], p0[:], b0_t[:])
                nc.scalar.copy(o1[:], p1[:])
                nc.vector.tensor_copy(o2[:], p2[:])
                nc.scalar.copy(o3[:], p3[:])

                nc.sync.dma_start(outT[0:128, sl], o0[:])
                nc.sync.dma_start(outT[128:256, sl], o1[:])
                nc.sync.dma_start(outT[256:384, sl], o2[:])
                nc.sync.dma_start(outT[384:480, sl], o3[:])

    nc.compile()
    return nc


def _get_program(npad):
    if npad not in _PROGRAM_CACHE:
        _PROGRAM_CACHE[npad] = _build_program(npad)
    return _PROGRAM_CACHE[npad]


def _repack_inputs(x1, x2):
    """Build XR [960, N]: channel-major, component-de-interleaved, both inputs."""
    n = x1.shape[0]
    xr = np.empty((960, n), dtype=np.float32)
    xr[0:128] = x1[:, 0:128].T
    xr[128:256] = x2[:, 0:128].T
    v1 = x1[:, 128:320].reshape(n, MUL1, 3)
    v2 = x2[:, 128:320].reshape(n, MUL1, 3)
    for i in range(3):
        base = 256 + 128 * i
        xr[base:base + 64] = v1[:, :, i].T
        xr[base + 64:base + 128] = v2[:, :, i].T
    t1 = x1[:, 320:480].reshape(n, MUL2, 5)
    t2 = x2[:, 320:480].reshape(n, MUL2, 5)
    for i in range(5):
        base = 640 + 64 * i
        xr[base:base + 32] = t1[:, :, i].T
        xr[base + 32:base + 64] = t2[:, :, i].T
    return xr


def _assemble_output(outs):
    """outs: list of 8 outT arrays [480, NPAD] -> full [N_TOTAL, 480]."""
    full = np.empty((N_TOTAL, 480), dtype=np.float32)
    for c, o in enumerate(outs):
        o = o[:, :NC_NODES]
        rows = slice(c * NC_NODES, (c + 1) * NC_NODES)
        full[rows, 0:128] = o[0:128].T
        full[rows, 128:320] = (
            o[128:320].reshape(3, MUL1, NC_NODES).transpose(2, 1, 0).reshape(NC_NODES, 192)
        )
        full[rows, 320:480] = (
            o[320:480].reshape(5, MUL2, NC_NODES).transpose(2, 1, 0).reshape(NC_NODES, 160)
        )
    return full


def kernel(x1, x2, W0, W1, W2, b0):
    from concourse.bass_utils import run_bass_kernel_spmd

    x1 = np.asarray(x1, dtype=np.float32)
    x2 = np.asarray(x2, dtype=np.float32)
    inv0 = np.float32(1.0 / np.sqrt(2 * MUL0))
    inv1 = np.float32(1.0 / np.sqrt(2 * MUL1))
    inv2 = np.float32(1.0 / np.sqrt(2 * MUL2))
    w0s = np.ascontiguousarray(np.asarray(W0, np.float32) * inv0)      # [256, 128]
    w1s = np.ascontiguousarray(np.asarray(W1, np.float32) * inv1)      # [128, 64]
    w2s = np.ascontiguousarray(np.asarray(W2, np.float32) * inv2)      # [64, 32]
    w2pair = np.zeros((128, 64), dtype=np.float32)                     # blockdiag(W2s, W2s)
    w2pair[0:64, 0:32] = w2s
    w2pair[64:128, 32:64] = w2s
    b0c = np.ascontiguousarray(np.asarray(b0, np.float32).reshape(128, 1))

    xr = _repack_inputs(x1, x2)

    weights = {
        "w0a": np.ascontiguousarray(w0s[0:128]),
        "w0b": np.ascontiguousarray(w0s[128:256]),
        "w1d": w1s,
        "w2p": w2pair,
        "w2s": w2s,
        "b0d": b0c,
    }
    in_maps = []
    for c in range(N_CORES):
        xrc = np.zeros((960, NPAD), dtype=np.float32)
        xrc[:, :NC_NODES] = xr[:, c * NC_NODES:(c + 1) * NC_NODES]
        in_maps.append({"xin": xrc, **weights})

    nc = _get_program(NPAD)
    res = run_bass_kernel_spmd(nc, in_maps, core_ids=list(range(N_CORES)))
    outs = [r["outT"] for r in res.results]
    return _assemble_output(outs)


# revision 2
# speedup vs baseline: 1.2883x; 1.2883x over previous
"""Trainium2 Bass kernel for the e3nn-style concat + per-irrep Linear problem.

Reference computation (N = 200000 nodes, 480-dim features per input):
  per input: 128x0e (dims 0:128) + 64x1e (dims 128:320) + 32x2e (dims 320:480)
  s = [s1, s2] @ W0 * inv0 + b0                   # [N, 128]
  v = einsum('nmi,mo->noi', [v1,v2], W1) * inv1   # [N, 64, 3]
  t = einsum('nmi,mo->noi', [t1,t2], W2) * inv2   # [N, 32, 5]
  out = concat([s, v.flat, t.flat], axis=1)       # [N, 480]

Strategy (memory-bound, data-parallel over nodes across 8 cores):
  - Host: repack both inputs into one channel-major tensor XR [1024, N]
    (irrep components de-interleaved into eight 128-row contraction slabs),
    fold the 1/sqrt(K) norms into the weights, pair up the five l=2
    components into block-diagonal weights.
  - Device: per 512-node block, two 1 MB chunked DMA loads, 8 fp32 matmuls
    with stationary weights (streaming 512 node-columns into PSUM),
    bias-add/copy PSUM->SBUF, one 1 MB chunked DMA store of the
    channel-major output outT [512, N].
  - Host: transpose/interleave outT back to the reference layout.
"""

import numpy as np

MUL0, MUL1, MUL2 = 128, 64, 32
N_TOTAL = 200000
N_CORES = 8
NC_NODES = N_TOTAL // N_CORES          # 25000
NODE_BLOCK = 512
N_BLOCKS = (NC_NODES + NODE_BLOCK - 1) // NODE_BLOCK   # 49
NPAD = N_BLOCKS * NODE_BLOCK           # 25088

_PROGRAM_CACHE = {}


def _build_program(npad):
    import concourse.mybir as mybir
    from concourse import bacc
    import concourse.tile as tile

    f32 = mybir.dt.float32
    NB = NODE_BLOCK
    nc = bacc.Bacc("TRN2", target_bir_lowering=False, debug=False)

    xin = nc.dram_tensor("xin", [1024, npad], f32, kind="ExternalInput").ap()
    w0a = nc.dram_tensor("w0a", [128, 128], f32, kind="ExternalInput").ap()
    w0b = nc.dram_tensor("w0b", [128, 128], f32, kind="ExternalInput").ap()
    w1d = nc.dram_tensor("w1d", [128, 64], f32, kind="ExternalInput").ap()
    w2p = nc.dram_tensor("w2p", [128, 64], f32, kind="ExternalInput").ap()
    w2s = nc.dram_tensor("w2s", [64, 32], f32, kind="ExternalInput").ap()
    b0d = nc.dram_tensor("b0d", [128, 1], f32, kind="ExternalInput").ap()
    outT = nc.dram_tensor("outT", [512, npad], f32, kind="ExternalOutput").ap()

    xin_c = xin.rearrange("(c p) n -> p c n", p=128)    # [128, 8, npad]
    outT_c = outT.rearrange("(c p) n -> p c n", p=128)  # [128, 4, npad]

    nblocks = npad // NB

    with tile.TileContext(nc) as tc:
        with (
            tc.tile_pool(name="wpool", bufs=1) as wpool,
            tc.tile_pool(name="inpool", bufs=4) as inpool,
            tc.tile_pool(name="psum", bufs=2, space="PSUM") as psum,
            tc.tile_pool(name="outpool", bufs=4) as outpool,
        ):
            wa_t = wpool.tile([128, 128], f32)
            wb_t = wpool.tile([128, 128], f32)
            w1_t = wpool.tile([128, 64], f32)
            w2p_t = wpool.tile([128, 64], f32)
            w2s_t = wpool.tile([64, 32], f32)
            b0_t = wpool.tile([128, 1], f32)
            nc.sync.dma_start(wa_t[:], w0a)
            nc.sync.dma_start(wb_t[:], w0b)
            nc.sync.dma_start(w1_t[:], w1d)
            nc.sync.dma_start(w2p_t[:], w2p)
            nc.sync.dma_start(w2s_t[:], w2s)
            nc.sync.dma_start(b0_t[:], b0d)

            for blk in range(nblocks):
                sl = slice(blk * NB, (blk + 1) * NB)

                # Chunked loads: tina <- slabs [s1, s2, v0, v1], tinb <- [v2, tp0, tp1, t4]
                tina = inpool.tile([128, 4 * NB], f32)
                tinb = inpool.tile([128, 4 * NB], f32)
                nc.sync.dma_start(
                    tina[:].rearrange("p (c n) -> p c n", c=4), xin_c[:, 0:4, sl]
                )
                nc.sync.dma_start(
                    tinb[:].rearrange("p (c n) -> p c n", c=4), xin_c[:, 4:8, sl]
                )

                # out^T row chunks: [s(128)] [v0|v1] [v2|t0,t1] [t2,t3|t4,junk]
                p0 = psum.tile([128, NB], f32)
                p1 = psum.tile([128, NB], f32)
                p2 = psum.tile([128, NB], f32)
                p3 = psum.tile([96, NB], f32)

                nc.tensor.matmul(p0[:], wa_t[:], tina[:, 0:NB], start=True, stop=False)
                nc.tensor.matmul(p0[:], wb_t[:], tina[:, NB:2 * NB], start=False, stop=True)
                nc.tensor.matmul(p1[0:64, :], w1_t[:], tina[:, 2 * NB:3 * NB])
                nc.tensor.matmul(p1[64:128, :], w1_t[:], tina[:, 3 * NB:4 * NB])
                nc.tensor.matmul(p2[0:64, :], w1_t[:], tinb[:, 0:NB])
                nc.tensor.matmul(p2[64:128, :], w2p_t[:], tinb[:, NB:2 * NB])
                nc.tensor.matmul(p3[0:64, :], w2p_t[:], tinb[:, 2 * NB:3 * NB])
                nc.tensor.matmul(p3[64:96, :], w2s_t[:], tinb[0:64, 3 * NB:4 * NB])

                tout = outpool.tile([128, 4 * NB], f32)
                nc.vector.tensor_scalar_add(tout[:, 0:NB], p0[:], b0_t[:])
                nc.scalar.copy(tout[:, NB:2 * NB], p1[:])
                nc.vector.tensor_copy(tout[:, 2 * NB:3 * NB], p2[:])
                nc.scalar.copy(tout[0:96, 3 * NB:4 * NB], p3[:])

                nc.scalar.dma_start(
                    outT_c[:, :, sl], tout[:].rearrange("p (c n) -> p c n", c=4)
                )

    nc.compile()
    return nc


def _get_program(npad):
    if npad not in _PROGRAM_CACHE:
        _PROGRAM_CACHE[npad] = _build_program(npad)
    return _PROGRAM_CACHE[npad]


def _repack_inputs(x1, x2):
    """Build XR [1024, N]: channel-major, component-de-interleaved, both inputs.

    Row slabs (128 rows each): [s1] [s2] [v1_0|v2_0] [v1_1|v2_1] [v1_2|v2_2]
    [t_0|t_1] [t_2|t_3] [t_4|zeros], each t_i = [t1_i(32); t2_i(32)].
    """
    n = x1.shape[0]
    xr = np.zeros((1024, n), dtype=np.float32)
    xr[0:128] = x1[:, 0:128].T
    xr[128:256] = x2[:, 0:128].T
    v1 = x1[:, 128:320].reshape(n, MUL1, 3)
    v2 = x2[:, 128:320].reshape(n, MUL1, 3)
    for i in range(3):
        base = 256 + 128 * i
        xr[base:base + 64] = v1[:, :, i].T
        xr[base + 64:base + 128] = v2[:, :, i].T
    t1 = x1[:, 320:480].reshape(n, MUL2, 5)
    t2 = x2[:, 320:480].reshape(n, MUL2, 5)
    for i in range(5):
        base = 640 + 64 * i
        xr[base:base + 32] = t1[:, :, i].T
        xr[base + 32:base + 64] = t2[:, :, i].T
    return xr


def _prepare_in_maps(x1, x2, W0, W1, W2, b0):
    x1 = np.asarray(x1, dtype=np.float32)
    x2 = np.asarray(x2, dtype=np.float32)
    inv0 = np.float32(1.0 / np.sqrt(2 * MUL0))
    inv1 = np.float32(1.0 / np.sqrt(2 * MUL1))
    inv2 = np.float32(1.0 / np.sqrt(2 * MUL2))
    w0s = np.asarray(W0, np.float32) * inv0                            # [256, 128]
    w1s = np.ascontiguousarray(np.asarray(W1, np.float32) * inv1)      # [128, 64]
    w2s = np.ascontiguousarray(np.asarray(W2, np.float32) * inv2)      # [64, 32]
    w2pair = np.zeros((128, 64), dtype=np.float32)                     # blockdiag(W2s, W2s)
    w2pair[0:64, 0:32] = w2s
    w2pair[64:128, 32:64] = w2s
    weights = {
        "w0a": np.ascontiguousarray(w0s[0:128]),
        "w0b": np.ascontiguousarray(w0s[128:256]),
        "w1d": w1s,
        "w2p": w2pair,
        "w2s": w2s,
        "b0d": np.ascontiguousarray(np.asarray(b0, np.float32).reshape(128, 1)),
    }
    xr = _repack_inputs(x1, x2)
    in_maps = []
    for c in range(N_CORES):
        xrc = np.zeros((1024, NPAD), dtype=np.float32)
        xrc[:, :NC_NODES] = xr[:, c * NC_NODES:(c + 1) * NC_NODES]
        in_maps.append({"xin": xrc, **weights})
    return in_maps


def _assemble_output(outs):
    """outs: list of 8 outT arrays [512, NPAD] -> full [N_TOTAL, 480]."""
    full = np.empty((N_TOTAL, 480), dtype=np.float32)
    for c, o in enumerate(outs):
        o = o[:480, :NC_NODES]
        rows = slice(c * NC_NODES, (c + 1) * NC_NODES)
        full[rows, 0:128] = o[0:128].T
        full[rows, 128:320] = (
            o[128:320].reshape(3, MUL1, NC_NODES).transpose(2, 1, 0).reshape(NC_NODES, 192)
        )
        full[rows, 320:480] = (
            o[320:480].reshape(5, MUL2, NC_NODES).transpose(2, 1, 0).reshape(NC_NODES, 160)
        )
    return full


def kernel(x1, x2, W0, W1, W2, b0):
    from concourse.bass_utils import run_bass_kernel_spmd

    in_maps = _prepare_in_maps(x1, x2, W0, W1, W2, b0)
    nc = _get_program(NPAD)
    res = run_bass_kernel_spmd(nc, in_maps, core_ids=list(range(N_CORES)))
    return _assemble_output([r["outT"] for r in res.results])
